# revision 19
# baseline (speedup 1.0000x reference)
"""Trainium2 Bass kernel for nn_GAT_87617332838818.

Mathematical collapse: the reference GAT aggregates ``alpha * hp[:, dst]``
over incoming edges per destination node.  Since the softmax weights alpha
sum to exactly 1 within each destination segment and the aggregated message
``hp[dst]`` is constant within the segment, the whole message-passing step
is the identity: ``out[n] = hp[n]``.  The network therefore reduces to a
per-node 3-layer MLP:

    logits = W2r @ elu(W1r @ elu(W0r @ x^T))        (per node column)

with W0r = W0.reshape(96,128), W1r = W1.reshape(96,96), W2r = W2.reshape(40,96)
(head-concat order matches the plain reshape).

Device strategy (8 NeuronCores, node-sharded 6250 rows each):
  - activations kept feature-on-partition: xT [128, n], h [96, n]
  - ELU via the split  elu(p') + 1 = max(p',0) + min(exp(p'),1)  with
    p' = p + nb (nb folds the "+1" inflation of the previous layer:
    nb = -W @ ones).  r = max(p+nb,0) and t = min(exp(p+nb),1) are fed
    through TWO accumulating matmuls (linearity), so the inflated h+1 is
    only ever formed in f32 PSUM — bf16-safe.
  - final layer bias cb2 = W2 @ ones subtracted in the output drain pass.
  - 512-column groups (one PSUM bank per matmul), 5-deep software
    pipeline (stage1/stage2 trail by 2 ticks so cross-engine dependency
    chains have slack and the tick period is engine-work-bound); L2
    outputs of a pair of groups packed vertically (rows 0:40 / 64:104)
    into one [104,512] PSUM tile -> one drain + one DMA.  The t-min
    runs once per PAIR over [96,1024] (DVE 4x amortizes overhead).
  - Engine cost model (measured): ACT pass (172+FD)/1.2 ns, DVE PSUM
    pass (120+FD)/0.96 ns, DVE bf16-SBUF 4x pass (58+FD/4)/0.96 ns,
    warm matmul N/2.4 ns.  exp must be on ACT; r-drains split ACT/DVE
    to balance (~9 of 26 on ACT); t and out drains on DVE.
  - Head (fixed ~5us: 3.3us engine init barrier + ~1.2us program
    TENSOR_LOADs before any user instruction): a dummy exp at the top
    of the Scalar queue pulls the ~2.7us ACT_TABLE_LOAD under the DMA
    wait; 8 junk matmuls on DVE-memset garbage cover the ~3.4us HAM
    clock warmup so the real matmuls run at 2.4 GHz (measured: without
    them 41/65 matmuls run at the cold 1.2 GHz rate).
  - Input DMAs split across the two parallel HWDGE rings in need-order
    (a ring's 2nd issue can block on its 1st DMA completing):
    sync: xb0 (w0+group0), wbb, xb2 (groups 6-12); scalar: xb1
    (groups 1-5).  No SWDGE — a gpsimd DMA at startup was measured
    wedging its ring ~7us.  Output DMAs ride sync, last pair smallest
    (106 cols) to minimize the final completion receipt.
  - Emission: per tick all matmuls first, then ACT, then DVE passes
    with fresh dependencies late in each FIFO queue (the Tile
    scheduler further reorders by dependency/priority).
  - Measured (8 cores, max-core HW exec): 40.3-40.7us vs 47.9us for
    the previous baseline; ACT is the binding engine (~73% busy incl
    sem checks; exp 15.3us + relu 4.9us), DVE ~66%; the mid-section
    sits at the elementwise cost floor with the r-drain ACT/DVE split
    balanced (shifting any pass between them is a wash).
  - NOTE: engine passes whose PSUM AP spans two banks crash the device
    (NRT_EXEC_UNIT_UNRECOVERABLE) — keep all PSUM APs within one bank.
    TRN2 matmul PSUM output must be fp32 (bf16 PSUM is TRN3-only).
"""

import os
import sys

import numpy as np

for _p in ("/root/.axon_site/_ro/trn_rl_repo", "/opt/trn_rl_repo"):
    if os.path.isdir(_p) and _p not in sys.path:
        sys.path.append(_p)

import concourse.bass as bass
import concourse.tile as tile
from concourse import bacc, mybir
from concourse.bass_utils import run_bass_kernel_spmd

N_CORES = 8
N_PER = 6250            # 50000 / 8
D_IN = 128
D_HID = 96
D_OUT = 40
FDP = 512               # group free-dim (1 PSUM bank)

F16 = mybir.dt.float16
BF16 = mybir.dt.bfloat16
F32 = mybir.dt.float32

Act = mybir.ActivationFunctionType
Alu = mybir.AluOpType

_pairs = [FDP] * (N_PER // FDP)
if N_PER % FDP:
    _pairs.append(N_PER % FDP)
P = len(_pairs)
_pstarts = [sum(_pairs[:i]) for i in range(P)]

# r-drain engine assignment: groups listed here drain on ACT, rest on DVE.
R0_ON_ACT = {1, 6, 11}
R1_ON_ACT = {0, 3, 8, 12}

# x batches: batch0 = w0 + group 0 (sync), batch1 = groups 1-5
# (scalar ring), batch2 = groups 6-12 (sync).  xw coordinates.
B0_COLS = D_HID + FDP                 # 608
B1_GROUPS = (1, 5)
B2_GROUPS = (6, P - 1)
B1_COLS = (D_HID + _pstarts[1], D_HID + _pstarts[5] + _pairs[5])
B2_COLS = (D_HID + _pstarts[6], D_HID + N_PER)
YT_COLS = ((P + 1) // 2 - 1) * FDP + _pairs[P - 1]   # 3178


def _build_program() -> bass.Bass:
    nc = bacc.Bacc(None, target_bir_lowering=False, debug=False)

    # xw packs [w0t | xT]: cols 0..95 = W0^T fp16, cols 96.. = x^T shard
    xw = nc.declare_dram_parameter("xw", [D_IN, D_HID + N_PER], F16,
                                   isOutput=False)
    # wbb packs [w1t | w2t | bias-bytes] bf16: cols 0:96 = W1^T, cols
    # 96:136 = W2^T (rows 0:96), cols 136:140 = two f32 bias columns
    # bit-packed as bf16 pairs (col 0 rows 0:96 = -(W1@1); col 1 rows
    # 0:40 & 64:104 = -(W2@1)).
    wbb = nc.declare_dram_parameter("wbb", [104, D_HID + D_OUT + 4], BF16,
                                    isOutput=False)
    # packed output: pair k at cols [512k, 512k+512): rows 0:40 = group 2k,
    # rows 64:104 = group 2k+1 (rows 40:64 unused). Host unpacks.
    yT = nc.declare_dram_parameter("yT", [104, YT_COLS], F16, isOutput=True)

    st = {}

    with tile.TileContext(nc) as tc:
        with (
            tc.tile_pool(name="consts", bufs=1) as consts,
            tc.tile_pool(name="xb0", bufs=1) as xb0p,
            tc.tile_pool(name="xb1", bufs=1) as xb1p,
            tc.tile_pool(name="xb2", bufs=1) as xb2p,
            tc.tile_pool(name="sb", bufs=4) as sb,
            tc.tile_pool(name="op", bufs=4) as opool,
            tc.tile_pool(name="ps0", bufs=3, space="PSUM") as ps0,
            tc.tile_pool(name="ps1", bufs=3, space="PSUM") as ps1,
            tc.tile_pool(name="ps2", bufs=2, space="PSUM") as ps2,
        ):
            # --- head schedule.
            xb0 = xb0p.tile([D_IN, B0_COLS], F16, tag="xb0")
            xb1 = xb1p.tile([D_IN, B1_COLS[1] - B1_COLS[0]], F16, tag="xb1")
            xb2 = xb2p.tile([D_IN, B2_COLS[1] - B2_COLS[0]], F16, tag="xb2")
            wbb_sb = consts.tile([104, D_HID + D_OUT + 4], BF16, tag="wbb")
            # DMA issue order is robust to per-ring FIFO serialization
            # (a ring's 2nd issue was observed blocking on the 1st DMA's
            # completion): sync ring xb0 -> wbb -> xb2 matches need order;
            # xb1 rides scalar's ring, issued before the dummy exp.
            nc.sync.dma_start(xb0[:], xw[:, 0:B0_COLS])
            nc.scalar.dma_start(xb1[:], xw[:, B1_COLS[0]:B1_COLS[1]])
            nc.sync.dma_start(wbb_sb[:], wbb[:])
            nc.sync.dma_start(xb2[:], xw[:, B2_COLS[0]:B2_COLS[1]])

            # Dummy exp pulls the ~2.7us ACT_TABLE_LOAD forward, done
            # before the first real exp (~9us).  memzero is a Copy-
            # activation via bitcast — no table needed.
            expd = consts.tile([1, 16], F32, tag="expd")
            nc.scalar.memzero(expd[:])
            nc.scalar.activation(expd[:], expd[:], Act.Exp)

            # Junk warmup matmuls: PE activity from ~5.5us (vector memset
            # feeds them right after program load) so the HAM clock gate
            # opens (~3.4us of activity) and stays open when the real
            # matmuls start at ~9us.  Measured: without these, 41/65
            # real matmuls run at the cold 1.2 GHz rate (+17% PE time).
            junk = consts.tile([D_IN, FDP + D_OUT], F16, tag="junk")
            nc.vector.memset(junk[:], 0.0)
            warm = ps0.tile([D_HID, FDP], F32, tag="p0", name="warm")
            for _ in range(8):
                nc.tensor.matmul(warm[:D_OUT], junk[:, FDP:FDP + D_OUT],
                                 junk[:, 0:FDP], start=True, stop=True)

            w0_sb = xb0[:, 0:D_HID]
            w1_sb = wbb_sb[:D_HID, :D_HID]
            w2_sb = wbb_sb[:D_HID, D_HID:D_HID + D_OUT]
            bias_f32 = wbb_sb[:, D_HID + D_OUT:D_HID + D_OUT + 4].bitcast(F32)
            nb1_sb = bias_f32[:D_HID, 0:1]
            ncb2d_sb = bias_f32[:104, 1:2]

            def xsrc(g):
                if g == 0:
                    return xb0, D_HID
                if B1_GROUPS[0] <= g <= B1_GROUPS[1]:
                    return xb1, D_HID + _pstarts[g] - B1_COLS[0]
                return xb2, D_HID + _pstarts[g] - B2_COLS[0]

            pair_state = {}
            pair_sb = {0: {}, 1: {}}   # layer -> pair idx -> e/t pair tiles

            def pair_tiles(lyr, g):
                """Pair-wide e/t tiles [96, 1024]; group g uses cols
                off:off+fd.  The t-min runs ONCE per pair over the full
                width (DVE 4x amortizes the fixed pass overhead)."""
                pr = pair_sb[lyr].setdefault(g // 2, {})
                if g % 2 == 0:
                    pr["e"] = sb.tile([D_HID, 2 * FDP], BF16,
                                      tag=f"e{lyr}", name=f"e{lyr}")
                    pr["t"] = sb.tile([D_HID, 2 * FDP], BF16,
                                      tag=f"t{lyr}", name=f"t{lyr}")
                return pr, (g % 2) * FDP

            def stage0_mm(g):
                fd = _pairs[g]
                xt, xo = xsrc(g)
                s = st.setdefault(g, {})
                s["p0"] = ps0.tile([D_HID, FDP], F32, tag="p0", name="p0")
                s["r0"] = sb.tile([D_HID, FDP], BF16, tag="r0", name="r0")
                s["pr0"], s["off0"] = pair_tiles(0, g)
                nc.tensor.matmul(s["p0"][:, :fd], w0_sb, xt[:, xo:xo + fd],
                                 start=True, stop=True)

            def stage1_mm(g):
                fd = _pairs[g]
                s = st[g]
                o0 = s["off0"]
                s["p1"] = ps1.tile([D_HID, FDP], F32, tag="p1", name="p1")
                s["r1"] = sb.tile([D_HID, FDP], BF16, tag="r1", name="r1")
                s["pr1"], s["off1"] = pair_tiles(1, g)
                nc.tensor.matmul(s["p1"][:, :fd], w1_sb,
                                 s.pop("r0")[:, :fd], start=True, stop=False)
                nc.tensor.matmul(s["p1"][:, :fd], w1_sb,
                                 s["pr0"]["t"][:, o0:o0 + fd],
                                 start=False, stop=True)

            def stage2_mm(g):
                fd = _pairs[g]
                s = st[g]
                if g % 2 == 0:
                    p2 = ps2.tile([104, FDP], F32, tag="p2")
                    pair_state[g // 2] = p2
                    rows = slice(0, D_OUT)
                else:
                    p2 = pair_state[g // 2]
                    rows = slice(64, 64 + D_OUT)
                o1 = s["off1"]
                nc.tensor.matmul(p2[rows, :fd], w2_sb, s.pop("r1")[:, :fd],
                                 start=True, stop=False)
                nc.tensor.matmul(p2[rows, :fd], w2_sb,
                                 s["pr1"]["t"][:, o1:o1 + fd],
                                 start=False, stop=True)

            def act0(g):
                fd = _pairs[g]
                s = st[g]
                o0 = s["off0"]
                nc.scalar.activation(s["pr0"]["e"][:, o0:o0 + fd],
                                     s["p0"][:, :fd], Act.Exp)
                if g in R0_ON_ACT:
                    nc.scalar.activation(s["r0"][:, :fd], s["p0"][:, :fd],
                                         Act.Relu)

            def act1(g):
                fd = _pairs[g]
                s = st[g]
                o1 = s["off1"]
                nc.scalar.activation(s["pr1"]["e"][:, o1:o1 + fd],
                                     s["p1"][:, :fd], Act.Exp, bias=nb1_sb)
                if g in R1_ON_ACT:
                    nc.scalar.activation(s["r1"][:, :fd], s["p1"][:, :fd],
                                         Act.Relu, bias=nb1_sb)

            def dve0_r(g):
                fd = _pairs[g]
                s = st[g]
                if g not in R0_ON_ACT:
                    nc.vector.tensor_scalar_max(s["r0"][:, :fd],
                                                s["p0"][:, :fd], 0.0)

            def dve1_r(g):
                fd = _pairs[g]
                s = st[g]
                if g not in R1_ON_ACT:
                    nc.vector.tensor_scalar(s["r1"][:, :fd], s["p1"][:, :fd],
                                            nb1_sb, 0.0, Alu.add, Alu.max)

            def dve0_t(g):
                # one pair-wide min per pair, after the odd group's exp.
                # Layer-0 mins run on the otherwise-idle GPSIMD (pure
                # bf16 SBUF->SBUF is legal there) to unload ACT+DVE.
                s = st[g]
                s.pop("p0")
                if (g % 2 == 1) or (g == P - 1):
                    pr = s["pr0"]
                    w = (g % 2) * FDP + _pairs[g]
                    nc.gpsimd.tensor_scalar_min(pr["t"][:, :w],
                                                pr["e"][:, :w], 1.0)

            def dve1_t(g):
                s = st[g]
                s.pop("p1")
                if (g % 2 == 1) or (g == P - 1):
                    pr = s["pr1"]
                    w = (g % 2) * FDP + _pairs[g]
                    nc.vector.tensor_scalar_min(pr["t"][:, :w],
                                                pr["e"][:, :w], 1.0)

            def out_drain(g):
                if not ((g % 2 == 1) or (g == P - 1)):
                    return
                fd = _pairs[g]
                st.pop(g - 1, None)
                st.pop(g, None)
                p2 = pair_state.pop(g // 2)
                nrows = 104 if g % 2 == 1 else D_OUT
                o = opool.tile([104, FDP], F16, tag="o")
                nc.vector.tensor_scalar_add(o[:nrows, :fd], p2[:nrows, :fd],
                                            ncb2d_sb[:nrows])
                kp = g // 2
                # final pair rides scalar's idle ring so its issue never
                # queues behind the previous pair's completion receipt
                eng = nc.scalar if g == P - 1 else nc.sync
                eng.dma_start(yT[:, kp * FDP:kp * FDP + fd], o[:, :fd])

            # 5-deep software-pipelined emission (stage1/stage2 trail by
            # 2 ticks each) so every cross-engine dependency chain
            # (exp0 -> t0 -> p1-mm -> exp1 ...) has 2 ticks of slack and
            # the tick period is engine-work-bound, not latency-bound.
            # Per tick: all matmuls first, then ACT passes, then DVE
            # passes with fresh dependencies late in each FIFO queue.
            for pp in range(P + 5):
                a, b, c = pp - 1, pp - 3, pp - 5
                if 0 <= a < P:
                    stage0_mm(a)
                if 0 <= b < P:
                    stage1_mm(b)
                if 0 <= c < P:
                    stage2_mm(c)
                if 0 <= a < P:
                    act0(a)
                if 0 <= b < P:
                    act1(b)
                if 0 <= a < P:
                    dve0_r(a)
                if 0 <= b < P:
                    dve1_r(b)
                if 0 <= a < P:
                    dve0_t(a)
                if 0 <= b < P:
                    dve1_t(b)
                if 0 <= c < P:
                    out_drain(c)

    nc.compile()
    return nc


_prog_cache = []
last_result = None


def kernel(**inputs) -> np.ndarray:
    global last_result
    x = np.asarray(inputs["x"], np.float32)           # [50000, 128]
    W0 = np.asarray(inputs["W0"], np.float32).reshape(D_HID, D_IN)
    W1 = np.asarray(inputs["W1"], np.float32).reshape(D_HID, D_HID)
    W2 = np.asarray(inputs["W2"], np.float32).reshape(D_OUT, D_HID)

    n = x.shape[0]
    assert n == N_CORES * N_PER, f"unexpected node count {n}"

    import ml_dtypes
    xT16 = x.T.astype(np.float16)                            # [128, 50000]
    w0t = W0.T.astype(np.float16)                            # [128, 96]
    w1tb = W1.T.astype(ml_dtypes.bfloat16)                   # [96, 96]
    w2tb = W2.T.astype(ml_dtypes.bfloat16)                   # [96, 40]
    biasm = np.zeros((104, 2), np.float32)
    biasm[:D_HID, 0] = -w1tb.astype(np.float32).sum(axis=0)  # -(W1 @ 1)
    ncb2 = -w2tb.astype(np.float32).sum(axis=0)              # -(W2 @ 1)
    biasm[:D_OUT, 1] = ncb2
    biasm[64:64 + D_OUT, 1] = ncb2                           # replicated
    wbbm = np.zeros((104, D_HID + D_OUT + 4), ml_dtypes.bfloat16)
    wbbm[:D_HID, :D_HID] = w1tb
    wbbm[:D_HID, D_HID:D_HID + D_OUT] = w2tb
    wbbm.view(np.uint16)[:, D_HID + D_OUT:] = \
        np.ascontiguousarray(biasm).view(np.uint16)

    if not _prog_cache:
        _prog_cache.append(_build_program())
    nc = _prog_cache[0]

    in_maps = []
    for i in range(N_CORES):
        xwi = np.ascontiguousarray(
            np.concatenate([w0t, xT16[:, i * N_PER:(i + 1) * N_PER]], axis=1))
        in_maps.append(dict(xw=xwi, wbb=wbbm))
    res = run_bass_kernel_spmd(nc, in_maps, list(range(N_CORES)))
    last_result = res
    out = np.empty((n, D_OUT), np.float32)
    for i in range(N_CORES):
        yt = np.asarray(res.results[i]["yT"], np.float32)  # [104, 3178]
        base = i * N_PER
        for kp in range((P + 1) // 2):
            c0 = kp * FDP
            g0 = 2 * kp
            w0_ = _pairs[g0]
            out[base + _pstarts[g0]:base + _pstarts[g0] + w0_] = \
                yt[0:D_OUT, c0:c0 + w0_].T
            if g0 + 1 < P:
                w1_ = _pairs[g0 + 1]
                out[base + _pstarts[g0 + 1]:base + _pstarts[g0 + 1] + w1_] = \
                    yt[64:64 + D_OUT, c0:c0 + w1_].T
    return out


if __name__ == "__main__":
    data = np.load("/tmp/gat_inputs.npz")
    y = kernel(**{k: data[k] for k in data.files})
    print("out", y.shape, y.dtype, "absmax", np.abs(y).max())


# revision 20
# speedup vs baseline: 2.3006x; 2.3006x over previous
"""Trainium2 Bass kernel for nn_GAT_87617332838818.

Mathematical collapse: the reference GAT aggregates ``alpha * hp[:, dst]``
over incoming edges per destination node.  Since the softmax weights alpha
sum to exactly 1 within each destination segment and the aggregated message
``hp[dst]`` is constant within the segment, the whole message-passing step
is the identity: ``out[n] = hp[n]``.  The network therefore reduces to a
per-node 3-layer MLP:

    logits = W2r @ elu(W1r @ elu(W0r @ x^T))        (per node column)

with W0r = W0.reshape(96,128), W1r = W1.reshape(96,96), W2r = W2.reshape(40,96)
(head-concat order matches the plain reshape).

Device strategy (8 NeuronCores, node-sharded 6250 rows each):
  - activations kept feature-on-partition: xT [128, n], h [96, n]
  - ELU via the split  elu(p') + 1 = max(p',0) + min(exp(p'),1)  with
    p' = p + nb (nb folds the "+1" inflation of the previous layer:
    nb = -W @ ones).  r = max(p+nb,0) and t = min(exp(p+nb),1) are fed
    through TWO accumulating matmuls (linearity), so the inflated h+1 is
    only ever formed in f32 PSUM — bf16-safe.
  - final layer bias cb2 = W2 @ ones subtracted in the output drain pass.
  - 512-column groups (one PSUM bank per matmul), 5-deep software
    pipeline (stage1/stage2 trail by 2 ticks so cross-engine dependency
    chains have slack and the tick period is engine-work-bound); L2
    outputs of a pair of groups packed vertically (rows 0:40 / 64:104)
    into one [104,512] PSUM tile -> one drain + one DMA.  The t-min
    runs once per PAIR over [96,1024] (DVE 4x amortizes overhead).
  - Engine cost model (measured): ACT pass (172+FD)/1.2 ns, DVE PSUM
    pass (120+FD)/0.96 ns, DVE bf16-SBUF 4x pass (58+FD/4)/0.96 ns,
    warm matmul N/2.4 ns.  exp must be on ACT; r-drains split ACT/DVE
    to balance (~9 of 26 on ACT); t and out drains on DVE.
  - Head (fixed ~5us: 3.3us engine init barrier + ~1.2us program
    TENSOR_LOADs before any user instruction): a dummy exp at the top
    of the Scalar queue pulls the ~2.7us ACT_TABLE_LOAD under the DMA
    wait; 8 junk matmuls on DVE-memset garbage cover the ~3.4us HAM
    clock warmup so the real matmuls run at 2.4 GHz (measured: without
    them 41/65 matmuls run at the cold 1.2 GHz rate).
  - Input DMAs split across the two parallel HWDGE rings in need-order
    (a ring's 2nd issue can block on its 1st DMA completing):
    sync: xb0 (w0+group0), wbb, xb2 (groups 6-12); scalar: xb1
    (groups 1-5).  No SWDGE — a gpsimd DMA at startup was measured
    wedging its ring ~7us.  Output DMAs ride sync, last pair smallest
    (106 cols) to minimize the final completion receipt.
  - Emission: per tick all matmuls first, then ACT, then DVE passes
    with fresh dependencies late in each FIFO queue (the Tile
    scheduler further reorders by dependency/priority).
  - Measured (8 cores, max-core HW exec): 40.3-40.7us vs 47.9us for
    the previous baseline; ACT is the binding engine (~73% busy incl
    sem checks; exp 15.3us + relu 4.9us), DVE ~66%; the mid-section
    sits at the elementwise cost floor with the r-drain ACT/DVE split
    balanced (shifting any pass between them is a wash).
  - NOTE: engine passes whose PSUM AP spans two banks crash the device
    (NRT_EXEC_UNIT_UNRECOVERABLE) — keep all PSUM APs within one bank.
    TRN2 matmul PSUM output must be fp32 (bf16 PSUM is TRN3-only).
"""

import os
import sys

import numpy as np

for _p in ("/root/.axon_site/_ro/trn_rl_repo", "/opt/trn_rl_repo"):
    if os.path.isdir(_p) and _p not in sys.path:
        sys.path.append(_p)

import concourse.bass as bass
import concourse.tile as tile
from concourse import bacc, mybir
from concourse.bass_utils import run_bass_kernel_spmd

N_CORES = 8
N_PER = 6250            # 50000 / 8
D_IN = 128
D_HID = 96
D_OUT = 40
FDP = 512               # group free-dim (1 PSUM bank)

F16 = mybir.dt.float16
BF16 = mybir.dt.bfloat16
F32 = mybir.dt.float32

Act = mybir.ActivationFunctionType
Alu = mybir.AluOpType

_pairs = [FDP] * (N_PER // FDP)
if N_PER % FDP:
    _pairs.append(N_PER % FDP)
P = len(_pairs)
_pstarts = [sum(_pairs[:i]) for i in range(P)]

# r-drain engine assignment: groups listed here drain on ACT, rest on DVE.
R0_ON_ACT = {1, 4, 7, 10}
R1_ON_ACT = {0, 3, 6, 9, 12}

# x batches: batch0 = w0 + group 0 (sync), batch1 = groups 1-5
# (scalar ring), batch2 = groups 6-12 (sync).  xw coordinates.
B0_COLS = D_HID + FDP                 # 608
B1_GROUPS = (1, 5)
B2_GROUPS = (6, P - 1)
B1_COLS = (D_HID + _pstarts[1], D_HID + _pstarts[5] + _pairs[5])
B2_COLS = (D_HID + _pstarts[6], D_HID + N_PER)
YT_COLS = ((P + 1) // 2 - 1) * FDP + _pairs[P - 1]   # 3178


def _build_program() -> bass.Bass:
    nc = bacc.Bacc(None, target_bir_lowering=False, debug=False)

    # xw packs [w0t | xT]: cols 0..95 = W0^T fp16, cols 96.. = x^T shard
    xw = nc.declare_dram_parameter("xw", [D_IN, D_HID + N_PER], F16,
                                   isOutput=False)
    # wbb packs [w1t | w2t | bias-bytes] bf16: cols 0:96 = W1^T, cols
    # 96:136 = W2^T (rows 0:96), cols 136:140 = two f32 bias columns
    # bit-packed as bf16 pairs (col 0 rows 0:96 = -(W1@1); col 1 rows
    # 0:40 & 64:104 = -(W2@1)).
    wbb = nc.declare_dram_parameter("wbb", [104, D_HID + D_OUT + 4], BF16,
                                    isOutput=False)
    # packed output: pair k at cols [512k, 512k+512): rows 0:40 = group 2k,
    # rows 64:104 = group 2k+1 (rows 40:64 unused). Host unpacks.
    yT = nc.declare_dram_parameter("yT", [104, YT_COLS], F16, isOutput=True)

    st = {}

    with tile.TileContext(nc) as tc:
        with (
            tc.tile_pool(name="consts", bufs=1) as consts,
            tc.tile_pool(name="xb0", bufs=1) as xb0p,
            tc.tile_pool(name="xb1", bufs=1) as xb1p,
            tc.tile_pool(name="xb2", bufs=1) as xb2p,
            tc.tile_pool(name="sb", bufs=4) as sb,
            tc.tile_pool(name="op", bufs=4) as opool,
            tc.tile_pool(name="ps0", bufs=3, space="PSUM") as ps0,
            tc.tile_pool(name="ps1", bufs=3, space="PSUM") as ps1,
            tc.tile_pool(name="ps2", bufs=2, space="PSUM") as ps2,
        ):
            # --- head schedule.
            xb0 = xb0p.tile([D_IN, B0_COLS], F16, tag="xb0")
            xb1 = xb1p.tile([D_IN, B1_COLS[1] - B1_COLS[0]], F16, tag="xb1")
            xb2 = xb2p.tile([D_IN, B2_COLS[1] - B2_COLS[0]], F16, tag="xb2")
            wbb_sb = consts.tile([104, D_HID + D_OUT + 4], BF16, tag="wbb")
            # DMA issue order is robust to per-ring FIFO serialization
            # (a ring's 2nd issue was observed blocking on the 1st DMA's
            # completion): sync ring xb0 -> wbb -> xb2 matches need order;
            # xb1 rides scalar's ring, issued before the dummy exp.
            nc.sync.dma_start(xb0[:], xw[:, 0:B0_COLS])
            nc.scalar.dma_start(xb1[:], xw[:, B1_COLS[0]:B1_COLS[1]])
            nc.sync.dma_start(wbb_sb[:], wbb[:])
            nc.sync.dma_start(xb2[:], xw[:, B2_COLS[0]:B2_COLS[1]])

            # Dummy exp pulls the ~2.7us ACT_TABLE_LOAD forward, done
            # before the first real exp (~9us).  memzero is a Copy-
            # activation via bitcast — no table needed.
            expd = consts.tile([1, 16], F32, tag="expd")
            nc.scalar.memzero(expd[:])
            nc.scalar.activation(expd[:], expd[:], Act.Exp)

            # Junk warmup matmuls: PE activity from ~5.5us (vector memset
            # feeds them right after program load) so the HAM clock gate
            # opens (~3.4us of activity) and stays open when the real
            # matmuls start at ~9us.  Measured: without these, 41/65
            # real matmuls run at the cold 1.2 GHz rate (+17% PE time).
            junk = consts.tile([D_IN, FDP + D_OUT], F16, tag="junk")
            nc.vector.memset(junk[:], 0.0)
            warm = ps0.tile([D_HID, FDP], F32, tag="p0", name="warm")
            for _ in range(8):
                nc.tensor.matmul(warm[:D_OUT], junk[:, FDP:FDP + D_OUT],
                                 junk[:, 0:FDP], start=True, stop=True)

            w0_sb = xb0[:, 0:D_HID]
            w1_sb = wbb_sb[:D_HID, :D_HID]
            w2_sb = wbb_sb[:D_HID, D_HID:D_HID + D_OUT]
            bias_f32 = wbb_sb[:, D_HID + D_OUT:D_HID + D_OUT + 4].bitcast(F32)
            nb1_sb = bias_f32[:D_HID, 0:1]
            ncb2d_sb = bias_f32[:104, 1:2]

            def xsrc(g):
                if g == 0:
                    return xb0, D_HID
                if B1_GROUPS[0] <= g <= B1_GROUPS[1]:
                    return xb1, D_HID + _pstarts[g] - B1_COLS[0]
                return xb2, D_HID + _pstarts[g] - B2_COLS[0]

            pair_state = {}
            pair_sb = {0: {}, 1: {}}   # layer -> pair idx -> e/t pair tiles

            def pair_tiles(lyr, g):
                """Pair-wide e/t tiles [96, 1024]; group g uses cols
                off:off+fd.  The t-min runs ONCE per pair over the full
                width (DVE 4x amortizes the fixed pass overhead)."""
                pr = pair_sb[lyr].setdefault(g // 2, {})
                if g % 2 == 0:
                    pr["e"] = sb.tile([D_HID, 2 * FDP], BF16,
                                      tag=f"e{lyr}", name=f"e{lyr}")
                    pr["t"] = sb.tile([D_HID, 2 * FDP], BF16,
                                      tag=f"t{lyr}", name=f"t{lyr}")
                return pr, (g % 2) * FDP

            def stage0_mm(g):
                fd = _pairs[g]
                xt, xo = xsrc(g)
                s = st.setdefault(g, {})
                s["p0"] = ps0.tile([D_HID, FDP], F32, tag="p0", name="p0")
                s["r0"] = sb.tile([D_HID, FDP], BF16, tag="r0", name="r0")
                s["pr0"], s["off0"] = pair_tiles(0, g)
                nc.tensor.matmul(s["p0"][:, :fd], w0_sb, xt[:, xo:xo + fd],
                                 start=True, stop=True)

            def stage1_mm(g):
                fd = _pairs[g]
                s = st[g]
                o0 = s["off0"]
                s["p1"] = ps1.tile([D_HID, FDP], F32, tag="p1", name="p1")
                s["r1"] = sb.tile([D_HID, FDP], BF16, tag="r1", name="r1")
                s["pr1"], s["off1"] = pair_tiles(1, g)
                nc.tensor.matmul(s["p1"][:, :fd], w1_sb,
                                 s.pop("r0")[:, :fd], start=True, stop=False)
                nc.tensor.matmul(s["p1"][:, :fd], w1_sb,
                                 s["pr0"]["t"][:, o0:o0 + fd],
                                 start=False, stop=True)

            def stage2_mm(g):
                fd = _pairs[g]
                s = st[g]
                if g % 2 == 0:
                    p2 = ps2.tile([104, FDP], F32, tag="p2")
                    pair_state[g // 2] = p2
                    rows = slice(0, D_OUT)
                else:
                    p2 = pair_state[g // 2]
                    rows = slice(64, 64 + D_OUT)
                o1 = s["off1"]
                nc.tensor.matmul(p2[rows, :fd], w2_sb, s.pop("r1")[:, :fd],
                                 start=True, stop=False)
                nc.tensor.matmul(p2[rows, :fd], w2_sb,
                                 s["pr1"]["t"][:, o1:o1 + fd],
                                 start=False, stop=True)

            def act0(g):
                fd = _pairs[g]
                s = st[g]
                o0 = s["off0"]
                nc.scalar.activation(s["pr0"]["e"][:, o0:o0 + fd],
                                     s["p0"][:, :fd], Act.Exp)
                if g in R0_ON_ACT:
                    nc.scalar.activation(s["r0"][:, :fd], s["p0"][:, :fd],
                                         Act.Relu)

            def act1(g):
                fd = _pairs[g]
                s = st[g]
                o1 = s["off1"]
                nc.scalar.activation(s["pr1"]["e"][:, o1:o1 + fd],
                                     s["p1"][:, :fd], Act.Exp, bias=nb1_sb)
                if g in R1_ON_ACT:
                    nc.scalar.activation(s["r1"][:, :fd], s["p1"][:, :fd],
                                         Act.Relu, bias=nb1_sb)

            def dve0_r(g):
                fd = _pairs[g]
                s = st[g]
                if g not in R0_ON_ACT:
                    nc.vector.tensor_scalar_max(s["r0"][:, :fd],
                                                s["p0"][:, :fd], 0.0)

            def dve1_r(g):
                fd = _pairs[g]
                s = st[g]
                if g not in R1_ON_ACT:
                    nc.vector.tensor_scalar(s["r1"][:, :fd], s["p1"][:, :fd],
                                            nb1_sb, 0.0, Alu.add, Alu.max)

            def dve0_t(g):
                # one pair-wide min per pair, after the odd group's exp
                s = st[g]
                s.pop("p0")
                if (g % 2 == 1) or (g == P - 1):
                    pr = s["pr0"]
                    w = (g % 2) * FDP + _pairs[g]
                    nc.vector.tensor_scalar_min(pr["t"][:, :w],
                                                pr["e"][:, :w], 1.0)

            def dve1_t(g):
                s = st[g]
                s.pop("p1")
                if (g % 2 == 1) or (g == P - 1):
                    pr = s["pr1"]
                    w = (g % 2) * FDP + _pairs[g]
                    nc.vector.tensor_scalar_min(pr["t"][:, :w],
                                                pr["e"][:, :w], 1.0)

            def out_drain(g):
                if not ((g % 2 == 1) or (g == P - 1)):
                    return
                fd = _pairs[g]
                st.pop(g - 1, None)
                st.pop(g, None)
                p2 = pair_state.pop(g // 2)
                nrows = 104 if g % 2 == 1 else D_OUT
                o = opool.tile([104, FDP], F16, tag="o")
                nc.vector.tensor_scalar_add(o[:nrows, :fd], p2[:nrows, :fd],
                                            ncb2d_sb[:nrows])
                kp = g // 2
                # final pair rides scalar's idle ring so its issue never
                # queues behind the previous pair's completion receipt
                eng = nc.scalar if g == P - 1 else nc.sync
                eng.dma_start(yT[:, kp * FDP:kp * FDP + fd], o[:, :fd])

            # 5-deep software-pipelined emission (stage1/stage2 trail by
            # 2 ticks each) so every cross-engine dependency chain
            # (exp0 -> t0 -> p1-mm -> exp1 ...) has 2 ticks of slack and
            # the tick period is engine-work-bound, not latency-bound.
            # Per tick: all matmuls first, then ACT passes, then DVE
            # passes with fresh dependencies late in each FIFO queue.
            for pp in range(P + 5):
                a, b, c = pp - 1, pp - 3, pp - 5
                if 0 <= a < P:
                    stage0_mm(a)
                if 0 <= b < P:
                    stage1_mm(b)
                if 0 <= c < P:
                    stage2_mm(c)
                if 0 <= a < P:
                    act0(a)
                if 0 <= b < P:
                    act1(b)
                if 0 <= a < P:
                    dve0_r(a)
                if 0 <= b < P:
                    dve1_r(b)
                if 0 <= a < P:
                    dve0_t(a)
                if 0 <= b < P:
                    dve1_t(b)
                if 0 <= c < P:
                    out_drain(c)

    nc.compile()
    return nc


_prog_cache = []
last_result = None


def kernel(**inputs) -> np.ndarray:
    global last_result
    x = np.asarray(inputs["x"], np.float32)           # [50000, 128]
    W0 = np.asarray(inputs["W0"], np.float32).reshape(D_HID, D_IN)
    W1 = np.asarray(inputs["W1"], np.float32).reshape(D_HID, D_HID)
    W2 = np.asarray(inputs["W2"], np.float32).reshape(D_OUT, D_HID)

    n = x.shape[0]
    assert n == N_CORES * N_PER, f"unexpected node count {n}"

    import ml_dtypes
    xT16 = x.T.astype(np.float16)                            # [128, 50000]
    w0t = W0.T.astype(np.float16)                            # [128, 96]
    w1tb = W1.T.astype(ml_dtypes.bfloat16)                   # [96, 96]
    w2tb = W2.T.astype(ml_dtypes.bfloat16)                   # [96, 40]
    biasm = np.zeros((104, 2), np.float32)
    biasm[:D_HID, 0] = -w1tb.astype(np.float32).sum(axis=0)  # -(W1 @ 1)
    ncb2 = -w2tb.astype(np.float32).sum(axis=0)              # -(W2 @ 1)
    biasm[:D_OUT, 1] = ncb2
    biasm[64:64 + D_OUT, 1] = ncb2                           # replicated
    wbbm = np.zeros((104, D_HID + D_OUT + 4), ml_dtypes.bfloat16)
    wbbm[:D_HID, :D_HID] = w1tb
    wbbm[:D_HID, D_HID:D_HID + D_OUT] = w2tb
    wbbm.view(np.uint16)[:, D_HID + D_OUT:] = \
        np.ascontiguousarray(biasm).view(np.uint16)

    if not _prog_cache:
        _prog_cache.append(_build_program())
    nc = _prog_cache[0]

    in_maps = []
    for i in range(N_CORES):
        xwi = np.ascontiguousarray(
            np.concatenate([w0t, xT16[:, i * N_PER:(i + 1) * N_PER]], axis=1))
        in_maps.append(dict(xw=xwi, wbb=wbbm))
    res = run_bass_kernel_spmd(nc, in_maps, list(range(N_CORES)))
    last_result = res
    out = np.empty((n, D_OUT), np.float32)
    for i in range(N_CORES):
        yt = np.asarray(res.results[i]["yT"], np.float32)  # [104, 3178]
        base = i * N_PER
        for kp in range((P + 1) // 2):
            c0 = kp * FDP
            g0 = 2 * kp
            w0_ = _pairs[g0]
            out[base + _pstarts[g0]:base + _pstarts[g0] + w0_] = \
                yt[0:D_OUT, c0:c0 + w0_].T
            if g0 + 1 < P:
                w1_ = _pairs[g0 + 1]
                out[base + _pstarts[g0 + 1]:base + _pstarts[g0 + 1] + w1_] = \
                    yt[64:64 + D_OUT, c0:c0 + w1_].T
    return out


if __name__ == "__main__":
    data = np.load("/tmp/gat_inputs.npz")
    y = kernel(**{k: data[k] for k in data.files})
    print("out", y.shape, y.dtype, "absmax", np.abs(y).max())


# revision 21
# speedup vs baseline: 2.5851x; 1.1236x over previous
"""Trainium2 Bass kernel for nn_GAT_87617332838818.

Mathematical collapse: the reference GAT aggregates ``alpha * hp[:, dst]``
over incoming edges per destination node.  Since the softmax weights alpha
sum to exactly 1 within each destination segment and the aggregated message
``hp[dst]`` is constant within the segment, the whole message-passing step
is the identity: ``out[n] = hp[n]``.  The network therefore reduces to a
per-node 3-layer MLP:

    logits = W2r @ elu(W1r @ elu(W0r @ x^T))        (per node column)

with W0r = W0.reshape(96,128), W1r = W1.reshape(96,96), W2r = W2.reshape(40,96)
(head-concat order matches the plain reshape).

Device strategy (8 NeuronCores, node-sharded 6250 rows each):
  - activations kept feature-on-partition: xT [128, n], h [96, n]
  - ELU via the split  elu(p') + 1 = max(p',0) + min(exp(p'),1)  with
    p' = p + nb (nb folds the "+1" inflation of the previous layer:
    nb = -W @ ones).  r = max(p+nb,0) and t = min(exp(p+nb),1) are fed
    through TWO accumulating matmuls (linearity), so the inflated h+1 is
    only ever formed in f32 PSUM — bf16-safe.
  - final layer bias cb2 = W2 @ ones subtracted in the output drain pass.
  - 512-column groups (one PSUM bank per matmul), 5-deep software
    pipeline (stage1/stage2 trail by 2 ticks so cross-engine dependency
    chains have slack and the tick period is engine-work-bound); L2
    outputs of a pair of groups packed vertically (rows 0:40 / 64:104)
    into one [104,512] PSUM tile -> one drain + one DMA.  The t-min
    runs once per PAIR over [96,1024] (DVE 4x amortizes overhead).
  - Engine cost model (measured): ACT pass (172+FD)/1.2 ns, DVE PSUM
    pass (120+FD)/0.96 ns, DVE bf16-SBUF 4x pass (58+FD/4)/0.96 ns,
    warm matmul N/2.4 ns.  exp must be on ACT; r-drains split ACT/DVE
    to balance (~9 of 26 on ACT); t and out drains on DVE.
  - Head (fixed ~5us: 3.3us engine init barrier + ~1.2us program
    TENSOR_LOADs before any user instruction): a dummy exp at the top
    of the Scalar queue pulls the ~2.7us ACT_TABLE_LOAD under the DMA
    wait; 8 junk matmuls on DVE-memset garbage cover the ~3.4us HAM
    clock warmup so the real matmuls run at 2.4 GHz (measured: without
    them 41/65 matmuls run at the cold 1.2 GHz rate).
  - Input DMAs split across the two parallel HWDGE rings in need-order
    (a ring's 2nd issue can block on its 1st DMA completing):
    sync: xb0 (w0+group0), wbb, xb2 (groups 6-12); scalar: xb1
    (groups 1-5).  No SWDGE — a gpsimd DMA at startup was measured
    wedging its ring ~7us.  Output DMAs ride sync, last pair smallest
    (106 cols) to minimize the final completion receipt.
  - Emission: per tick all matmuls first, then ACT, then DVE passes
    with fresh dependencies late in each FIFO queue (the Tile
    scheduler further reorders by dependency/priority).
  - Measured (8 cores, max-core HW exec): 40.3-40.7us vs 47.9us for
    the previous baseline; ACT is the binding engine (~73% busy incl
    sem checks; exp 15.3us + relu 4.9us), DVE ~66%; the mid-section
    sits at the elementwise cost floor with the r-drain ACT/DVE split
    balanced (shifting any pass between them is a wash).
  - NOTE: engine passes whose PSUM AP spans two banks crash the device
    (NRT_EXEC_UNIT_UNRECOVERABLE) — keep all PSUM APs within one bank.
    TRN2 matmul PSUM output must be fp32 (bf16 PSUM is TRN3-only).
"""

import os
import sys

import numpy as np

for _p in ("/root/.axon_site/_ro/trn_rl_repo", "/opt/trn_rl_repo"):
    if os.path.isdir(_p) and _p not in sys.path:
        sys.path.append(_p)

import concourse.bass as bass
import concourse.tile as tile
from concourse import bacc, mybir
from concourse.bass_utils import run_bass_kernel_spmd

N_CORES = 8
N_PER = 6250            # 50000 / 8
D_IN = 128
D_HID = 96
D_OUT = 40
FDP = 512               # group free-dim (1 PSUM bank)

F16 = mybir.dt.float16
BF16 = mybir.dt.bfloat16
F32 = mybir.dt.float32

Act = mybir.ActivationFunctionType
Alu = mybir.AluOpType

_pairs = [FDP] * (N_PER // FDP)
if N_PER % FDP:
    _pairs.append(N_PER % FDP)
P = len(_pairs)
_pstarts = [sum(_pairs[:i]) for i in range(P)]

# r-drain engine assignment: groups listed here drain on ACT, rest on
# DVE.  LP-optimal split from measured pass costs (r1: DVE 697ns vs ACT
# 549; r0: 581 vs 549; out: 637 vs 570): all r0 on DVE, 8 of 13 r1 on
# ACT balances both engines at ~19.6us of mid-section work.
R0_ON_ACT = set()
R1_ON_ACT = {0, 2, 4, 5, 7, 9, 11, 12}

# x batches: batch0 = w0 + group 0 (sync), batch1 = groups 1-5
# (scalar ring), batch2 = groups 6-12 (sync).  xw coordinates.
B0_COLS = D_HID + FDP                 # 608
B1_GROUPS = (1, 5)
B2_GROUPS = (6, P - 1)
B1_COLS = (D_HID + _pstarts[1], D_HID + _pstarts[5] + _pairs[5])
B2_COLS = (D_HID + _pstarts[6], D_HID + N_PER)
YT_COLS = ((P + 1) // 2 - 1) * FDP + _pairs[P - 1]   # 3178


def _build_program() -> bass.Bass:
    nc = bacc.Bacc(None, target_bir_lowering=False, debug=False)

    # xw packs [w0t | xT]: cols 0..95 = W0^T fp16, cols 96.. = x^T shard
    xw = nc.declare_dram_parameter("xw", [D_IN, D_HID + N_PER], F16,
                                   isOutput=False)
    # wbb packs [w1t | w2t | bias-bytes] bf16: cols 0:96 = W1^T, cols
    # 96:136 = W2^T (rows 0:96), cols 136:140 = two f32 bias columns
    # bit-packed as bf16 pairs (col 0 rows 0:96 = -(W1@1); col 1 rows
    # 0:40 & 64:104 = -(W2@1)).
    wbb = nc.declare_dram_parameter("wbb", [104, D_HID + D_OUT + 4], BF16,
                                    isOutput=False)
    # packed output: pair k at cols [512k, 512k+512): rows 0:40 = group 2k,
    # rows 64:104 = group 2k+1 (rows 40:64 unused). Host unpacks.
    yT = nc.declare_dram_parameter("yT", [104, YT_COLS], F16, isOutput=True)

    st = {}

    with tile.TileContext(nc) as tc:
        with (
            tc.tile_pool(name="consts", bufs=1) as consts,
            tc.tile_pool(name="xb0", bufs=1) as xb0p,
            tc.tile_pool(name="xb1", bufs=1) as xb1p,
            tc.tile_pool(name="xb2", bufs=1) as xb2p,
            tc.tile_pool(name="sb", bufs=4) as sb,
            tc.tile_pool(name="op", bufs=4) as opool,
            tc.tile_pool(name="ps0", bufs=3, space="PSUM") as ps0,
            tc.tile_pool(name="ps1", bufs=3, space="PSUM") as ps1,
            tc.tile_pool(name="ps2", bufs=2, space="PSUM") as ps2,
        ):
            # --- head schedule.
            xb0 = xb0p.tile([D_IN, B0_COLS], F16, tag="xb0")
            xb1 = xb1p.tile([D_IN, B1_COLS[1] - B1_COLS[0]], F16, tag="xb1")
            xb2 = xb2p.tile([D_IN, B2_COLS[1] - B2_COLS[0]], F16, tag="xb2")
            wbb_sb = consts.tile([104, D_HID + D_OUT + 4], BF16, tag="wbb")
            # DMA issue order is robust to per-ring FIFO serialization
            # (a ring's 2nd issue was observed blocking on the 1st DMA's
            # completion): sync ring xb0 -> wbb -> xb2 matches need order;
            # xb1 rides scalar's ring, issued before the dummy exp.
            nc.sync.dma_start(xb0[:], xw[:, 0:B0_COLS])
            nc.scalar.dma_start(xb1[:], xw[:, B1_COLS[0]:B1_COLS[1]])
            nc.sync.dma_start(wbb_sb[:], wbb[:])
            nc.sync.dma_start(xb2[:], xw[:, B2_COLS[0]:B2_COLS[1]])

            # Dummy exp pulls the ~2.7us ACT_TABLE_LOAD forward, done
            # before the first real exp (~9us).  memzero is a Copy-
            # activation via bitcast — no table needed.
            expd = consts.tile([1, 16], F32, tag="expd")
            nc.scalar.memzero(expd[:])
            nc.scalar.activation(expd[:], expd[:], Act.Exp)

            # Junk warmup matmuls: PE activity from ~5.5us (vector memset
            # feeds them right after program load) so the HAM clock gate
            # opens (~3.4us of activity) and stays open when the real
            # matmuls start at ~9us.  Measured: without these, 41/65
            # real matmuls run at the cold 1.2 GHz rate (+17% PE time).
            junk = consts.tile([D_IN, FDP + D_OUT], F16, tag="junk")
            nc.vector.memset(junk[:], 0.0)
            warm = ps0.tile([D_HID, FDP], F32, tag="p0", name="warm")
            for _ in range(8):
                nc.tensor.matmul(warm[:D_OUT], junk[:, FDP:FDP + D_OUT],
                                 junk[:, 0:FDP], start=True, stop=True)

            w0_sb = xb0[:, 0:D_HID]
            w1_sb = wbb_sb[:D_HID, :D_HID]
            w2_sb = wbb_sb[:D_HID, D_HID:D_HID + D_OUT]
            bias_f32 = wbb_sb[:, D_HID + D_OUT:D_HID + D_OUT + 4].bitcast(F32)
            nb1_sb = bias_f32[:D_HID, 0:1]
            ncb2d_sb = bias_f32[:104, 1:2]

            def xsrc(g):
                if g == 0:
                    return xb0, D_HID
                if B1_GROUPS[0] <= g <= B1_GROUPS[1]:
                    return xb1, D_HID + _pstarts[g] - B1_COLS[0]
                return xb2, D_HID + _pstarts[g] - B2_COLS[0]

            pair_state = {}
            pair_sb = {0: {}, 1: {}}   # layer -> pair idx -> e/t pair tiles

            def pair_tiles(lyr, g):
                """Pair-wide e/t tiles [96, 1024]; group g uses cols
                off:off+fd.  The t-min runs ONCE per pair over the full
                width (DVE 4x amortizes the fixed pass overhead)."""
                pr = pair_sb[lyr].setdefault(g // 2, {})
                if g % 2 == 0:
                    pr["e"] = sb.tile([D_HID, 2 * FDP], BF16,
                                      tag=f"e{lyr}", name=f"e{lyr}")
                    pr["t"] = sb.tile([D_HID, 2 * FDP], BF16,
                                      tag=f"t{lyr}", name=f"t{lyr}")
                return pr, (g % 2) * FDP

            def stage0_mm(g):
                fd = _pairs[g]
                xt, xo = xsrc(g)
                s = st.setdefault(g, {})
                s["p0"] = ps0.tile([D_HID, FDP], F32, tag="p0", name="p0")
                s["r0"] = sb.tile([D_HID, FDP], BF16, tag="r0", name="r0")
                s["pr0"], s["off0"] = pair_tiles(0, g)
                nc.tensor.matmul(s["p0"][:, :fd], w0_sb, xt[:, xo:xo + fd],
                                 start=True, stop=True)

            def stage1_mm(g):
                fd = _pairs[g]
                s = st[g]
                o0 = s["off0"]
                s["p1"] = ps1.tile([D_HID, FDP], F32, tag="p1", name="p1")
                s["r1"] = sb.tile([D_HID, FDP], BF16, tag="r1", name="r1")
                s["pr1"], s["off1"] = pair_tiles(1, g)
                nc.tensor.matmul(s["p1"][:, :fd], w1_sb,
                                 s.pop("r0")[:, :fd], start=True, stop=False)
                nc.tensor.matmul(s["p1"][:, :fd], w1_sb,
                                 s["pr0"]["t"][:, o0:o0 + fd],
                                 start=False, stop=True)

            def stage2_mm(g):
                fd = _pairs[g]
                s = st[g]
                if g % 2 == 0:
                    p2 = ps2.tile([104, FDP], F32, tag="p2")
                    pair_state[g // 2] = p2
                    rows = slice(0, D_OUT)
                else:
                    p2 = pair_state[g // 2]
                    rows = slice(64, 64 + D_OUT)
                o1 = s["off1"]
                nc.tensor.matmul(p2[rows, :fd], w2_sb, s.pop("r1")[:, :fd],
                                 start=True, stop=False)
                nc.tensor.matmul(p2[rows, :fd], w2_sb,
                                 s["pr1"]["t"][:, o1:o1 + fd],
                                 start=False, stop=True)

            def act0(g):
                fd = _pairs[g]
                s = st[g]
                o0 = s["off0"]
                nc.scalar.activation(s["pr0"]["e"][:, o0:o0 + fd],
                                     s["p0"][:, :fd], Act.Exp)
                if g in R0_ON_ACT:
                    nc.scalar.activation(s["r0"][:, :fd], s["p0"][:, :fd],
                                         Act.Relu)

            def act1(g):
                fd = _pairs[g]
                s = st[g]
                o1 = s["off1"]
                nc.scalar.activation(s["pr1"]["e"][:, o1:o1 + fd],
                                     s["p1"][:, :fd], Act.Exp, bias=nb1_sb)
                if g in R1_ON_ACT:
                    nc.scalar.activation(s["r1"][:, :fd], s["p1"][:, :fd],
                                         Act.Relu, bias=nb1_sb)

            def dve0_r(g):
                fd = _pairs[g]
                s = st[g]
                if g not in R0_ON_ACT:
                    nc.vector.tensor_scalar_max(s["r0"][:, :fd],
                                                s["p0"][:, :fd], 0.0)

            def dve1_r(g):
                fd = _pairs[g]
                s = st[g]
                if g not in R1_ON_ACT:
                    nc.vector.tensor_scalar(s["r1"][:, :fd], s["p1"][:, :fd],
                                            nb1_sb, 0.0, Alu.add, Alu.max)

            def dve0_t(g):
                # one pair-wide min per pair, after the odd group's exp
                s = st[g]
                s.pop("p0")
                if (g % 2 == 1) or (g == P - 1):
                    pr = s["pr0"]
                    w = (g % 2) * FDP + _pairs[g]
                    nc.vector.tensor_scalar_min(pr["t"][:, :w],
                                                pr["e"][:, :w], 1.0)

            def dve1_t(g):
                s = st[g]
                s.pop("p1")
                if (g % 2 == 1) or (g == P - 1):
                    pr = s["pr1"]
                    w = (g % 2) * FDP + _pairs[g]
                    nc.vector.tensor_scalar_min(pr["t"][:, :w],
                                                pr["e"][:, :w], 1.0)

            def out_drain(g):
                if not ((g % 2 == 1) or (g == P - 1)):
                    return
                fd = _pairs[g]
                st.pop(g - 1, None)
                st.pop(g, None)
                p2 = pair_state.pop(g // 2)
                nrows = 104 if g % 2 == 1 else D_OUT
                o = opool.tile([104, FDP], F16, tag="o")
                nc.vector.tensor_scalar_add(o[:nrows, :fd], p2[:nrows, :fd],
                                            ncb2d_sb[:nrows])
                kp = g // 2
                # final pair rides scalar's idle ring so its issue never
                # queues behind the previous pair's completion receipt
                eng = nc.scalar if g == P - 1 else nc.sync
                eng.dma_start(yT[:, kp * FDP:kp * FDP + fd], o[:, :fd])

            # 5-deep software-pipelined emission (stage1/stage2 trail by
            # 2 ticks each) so every cross-engine dependency chain
            # (exp0 -> t0 -> p1-mm -> exp1 ...) has 2 ticks of slack and
            # the tick period is engine-work-bound, not latency-bound.
            # Per tick: all matmuls first, then ACT passes, then DVE
            # passes with fresh dependencies late in each FIFO queue.
            for pp in range(P + 5):
                a, b, c = pp - 1, pp - 3, pp - 5
                if 0 <= a < P:
                    stage0_mm(a)
                if 0 <= b < P:
                    stage1_mm(b)
                if 0 <= c < P:
                    stage2_mm(c)
                if 0 <= a < P:
                    act0(a)
                if 0 <= b < P:
                    act1(b)
                if 0 <= a < P:
                    dve0_r(a)
                if 0 <= b < P:
                    dve1_r(b)
                if 0 <= a < P:
                    dve0_t(a)
                if 0 <= b < P:
                    dve1_t(b)
                if 0 <= c < P:
                    out_drain(c)

    nc.compile()
    return nc


_prog_cache = []
last_result = None


def kernel(**inputs) -> np.ndarray:
    global last_result
    x = np.asarray(inputs["x"], np.float32)           # [50000, 128]
    W0 = np.asarray(inputs["W0"], np.float32).reshape(D_HID, D_IN)
    W1 = np.asarray(inputs["W1"], np.float32).reshape(D_HID, D_HID)
    W2 = np.asarray(inputs["W2"], np.float32).reshape(D_OUT, D_HID)

    n = x.shape[0]
    assert n == N_CORES * N_PER, f"unexpected node count {n}"

    import ml_dtypes
    xT16 = x.T.astype(np.float16)                            # [128, 50000]
    w0t = W0.T.astype(np.float16)                            # [128, 96]
    w1tb = W1.T.astype(ml_dtypes.bfloat16)                   # [96, 96]
    w2tb = W2.T.astype(ml_dtypes.bfloat16)                   # [96, 40]
    biasm = np.zeros((104, 2), np.float32)
    biasm[:D_HID, 0] = -w1tb.astype(np.float32).sum(axis=0)  # -(W1 @ 1)
    ncb2 = -w2tb.astype(np.float32).sum(axis=0)              # -(W2 @ 1)
    biasm[:D_OUT, 1] = ncb2
    biasm[64:64 + D_OUT, 1] = ncb2                           # replicated
    wbbm = np.zeros((104, D_HID + D_OUT + 4), ml_dtypes.bfloat16)
    wbbm[:D_HID, :D_HID] = w1tb
    wbbm[:D_HID, D_HID:D_HID + D_OUT] = w2tb
    wbbm.view(np.uint16)[:, D_HID + D_OUT:] = \
        np.ascontiguousarray(biasm).view(np.uint16)

    if not _prog_cache:
        _prog_cache.append(_build_program())
    nc = _prog_cache[0]

    in_maps = []
    for i in range(N_CORES):
        xwi = np.ascontiguousarray(
            np.concatenate([w0t, xT16[:, i * N_PER:(i + 1) * N_PER]], axis=1))
        in_maps.append(dict(xw=xwi, wbb=wbbm))
    res = run_bass_kernel_spmd(nc, in_maps, list(range(N_CORES)))
    last_result = res
    out = np.empty((n, D_OUT), np.float32)
    for i in range(N_CORES):
        yt = np.asarray(res.results[i]["yT"], np.float32)  # [104, 3178]
        base = i * N_PER
        for kp in range((P + 1) // 2):
            c0 = kp * FDP
            g0 = 2 * kp
            w0_ = _pairs[g0]
            out[base + _pstarts[g0]:base + _pstarts[g0] + w0_] = \
                yt[0:D_OUT, c0:c0 + w0_].T
            if g0 + 1 < P:
                w1_ = _pairs[g0 + 1]
                out[base + _pstarts[g0 + 1]:base + _pstarts[g0 + 1] + w1_] = \
                    yt[64:64 + D_OUT, c0:c0 + w1_].T
    return out


if __name__ == "__main__":
    data = np.load("/tmp/gat_inputs.npz")
    y = kernel(**{k: data[k] for k in data.files})
    print("out", y.shape, y.dtype, "absmax", np.abs(y).max())


# revision 23
# speedup vs baseline: 2.9161x; 1.1280x over previous
"""Trainium2 Bass kernel for nn_GAT_87617332838818.

Mathematical collapse: the reference GAT aggregates ``alpha * hp[:, dst]``
over incoming edges per destination node.  Since the softmax weights alpha
sum to exactly 1 within each destination segment and the aggregated message
``hp[dst]`` is constant within the segment, the whole message-passing step
is the identity: ``out[n] = hp[n]``.  The network therefore reduces to a
per-node 3-layer MLP:

    logits = W2r @ elu(W1r @ elu(W0r @ x^T))        (per node column)

with W0r = W0.reshape(96,128), W1r = W1.reshape(96,96), W2r = W2.reshape(40,96)
(head-concat order matches the plain reshape).

Device strategy (8 NeuronCores, node-sharded 6250 rows each):
  - activations kept feature-on-partition: xT [128, n], h [96, n]
  - ELU via the split  elu(p') + 1 = max(p',0) + min(exp(p'),1)  with
    p' = p + nb (nb folds the "+1" inflation of the previous layer:
    nb = -W @ ones).  r = max(p+nb,0) and t = min(exp(p+nb),1) are fed
    through TWO accumulating matmuls (linearity), so the inflated h+1 is
    only ever formed in f32 PSUM — bf16-safe.
  - final layer bias cb2 = W2 @ ones subtracted in the output drain pass.
  - 512-column groups (one PSUM bank per matmul), 5-deep software
    pipeline (stage1/stage2 trail by 2 ticks so cross-engine dependency
    chains have slack and the tick period is engine-work-bound); L2
    outputs of a pair of groups packed vertically (rows 0:40 / 64:104)
    into one [104,512] PSUM tile -> one drain + one DMA.  The t-min
    runs once per PAIR over [96,1024] (DVE 4x amortizes overhead).
  - Engine cost model (measured): ACT pass (172+FD)/1.2 ns, DVE PSUM
    pass (120+FD)/0.96 ns, DVE bf16-SBUF 4x pass (58+FD/4)/0.96 ns,
    warm matmul N/2.4 ns.  exp must be on ACT; r-drains LP-optimally
    split (all r0 on DVE, 8/13 r1 on ACT); t and out drains on DVE.
  - Head (fixed ~5us: 3.3us engine init barrier + ~1.2us program
    TENSOR_LOADs before any user instruction): a dummy exp at the top
    of the Scalar queue pulls the ~2.7us ACT_TABLE_LOAD under the DMA
    wait; 8 junk matmuls on DVE-memset garbage cover the ~3.4us HAM
    clock warmup so the real matmuls run at 2.4 GHz (measured: without
    them 41/65 matmuls run at the cold 1.2 GHz rate).
  - Input DMAs split across the two parallel HWDGE rings in need-order
    (a ring's 2nd issue can block on its 1st DMA completing):
    sync: xb0 (w0+group0), wbb, xb2 (groups 6-12); scalar: xb1
    (groups 1-5).  No SWDGE — a gpsimd DMA at startup was measured
    wedging its ring ~7us.  Output DMAs ride sync, last pair smallest
    (106 cols) to minimize the final completion receipt.
  - Emission: per tick all matmuls first, then ACT, then DVE passes
    with fresh dependencies late in each FIFO queue (the Tile
    scheduler further reorders by dependency/priority).
  - Measured (8 cores, max-core HW exec): 40.3-40.7us vs 47.9us for
    the previous baseline; ACT is the binding engine (~73% busy incl
    sem checks; exp 15.3us + relu 4.9us), DVE ~66%; the mid-section
    sits at the elementwise cost floor with the r-drain ACT/DVE split
    balanced (shifting any pass between them is a wash).
  - NOTE: engine passes whose PSUM AP spans two banks crash the device
    (NRT_EXEC_UNIT_UNRECOVERABLE) — keep all PSUM APs within one bank.
    TRN2 matmul PSUM output must be fp32 (bf16 PSUM is TRN3-only).
"""

import os
import sys

import numpy as np

for _p in ("/root/.axon_site/_ro/trn_rl_repo", "/opt/trn_rl_repo"):
    if os.path.isdir(_p) and _p not in sys.path:
        sys.path.append(_p)

import concourse.bass as bass
import concourse.tile as tile
from concourse import bacc, mybir
from concourse.bass_utils import run_bass_kernel_spmd

N_CORES = 8
N_PER = 6250            # 50000 / 8
D_IN = 128
D_HID = 96
D_OUT = 40
FDP = 512               # group free-dim (1 PSUM bank)

F16 = mybir.dt.float16
BF16 = mybir.dt.bfloat16
F32 = mybir.dt.float32

Act = mybir.ActivationFunctionType
Alu = mybir.AluOpType

_pairs = [FDP] * (N_PER // FDP)
if N_PER % FDP:
    _pairs.append(N_PER % FDP)
P = len(_pairs)
_pstarts = [sum(_pairs[:i]) for i in range(P)]

# r-drain engine assignment: groups listed here drain on ACT, rest on
# DVE.  LP-optimal split from measured pass costs (r1: DVE 697ns vs ACT
# 549; r0: 581 vs 549; out: 637 vs 570): all r0 on DVE, 8 of 13 r1 on
# ACT balances both engines at ~19.6us of mid-section work.
R0_ON_ACT = set()
R1_ON_ACT = {0, 2, 4, 5, 7, 9, 11, 12}

# x batches: batch0 = w0 + group 0 (sync), batch1 = groups 1-5
# (scalar ring), batch2 = groups 6-12 (sync).  xw coordinates.
B0_COLS = D_HID + FDP                 # 608
B1_GROUPS = (1, 5)
B2_GROUPS = (6, P - 1)
B1_COLS = (D_HID + _pstarts[1], D_HID + _pstarts[5] + _pairs[5])
B2_COLS = (D_HID + _pstarts[6], D_HID + N_PER)
YT_COLS = ((P + 1) // 2 - 1) * FDP + _pairs[P - 1]   # 3178


def _build_program() -> bass.Bass:
    nc = bacc.Bacc(None, target_bir_lowering=False, debug=False)

    # xw packs [w0t | xT]: cols 0..95 = W0^T fp16, cols 96.. = x^T shard
    xw = nc.declare_dram_parameter("xw", [D_IN, D_HID + N_PER], F16,
                                   isOutput=False)
    # wbb packs [w1t | w2t | bias-bytes] bf16: cols 0:96 = W1^T, cols
    # 96:136 = W2^T (rows 0:96), cols 136:140 = two f32 bias columns
    # bit-packed as bf16 pairs (col 0 rows 0:96 = -(W1@1); col 1 rows
    # 0:40 & 64:104 = -(W2@1)).
    wbb = nc.declare_dram_parameter("wbb", [104, D_HID + D_OUT + 4], BF16,
                                    isOutput=False)
    # packed output: pair k at cols [512k, 512k+512): rows 0:40 = group 2k,
    # rows 64:104 = group 2k+1 (rows 40:64 unused). Host unpacks.
    yT = nc.declare_dram_parameter("yT", [104, YT_COLS], F16, isOutput=True)

    st = {}

    with tile.TileContext(nc) as tc:
        with (
            tc.tile_pool(name="consts", bufs=1) as consts,
            tc.tile_pool(name="xb0", bufs=1) as xb0p,
            tc.tile_pool(name="xb1", bufs=1) as xb1p,
            tc.tile_pool(name="xb2", bufs=1) as xb2p,
            tc.tile_pool(name="sb", bufs=4) as sb,
            tc.tile_pool(name="op", bufs=4) as opool,
            tc.tile_pool(name="ps0", bufs=3, space="PSUM") as ps0,
            tc.tile_pool(name="ps1", bufs=3, space="PSUM") as ps1,
            tc.tile_pool(name="ps2", bufs=2, space="PSUM") as ps2,
        ):
            # --- head schedule.
            xb0 = xb0p.tile([D_IN, B0_COLS], F16, tag="xb0")
            xb1 = xb1p.tile([D_IN, B1_COLS[1] - B1_COLS[0]], F16, tag="xb1")
            xb2 = xb2p.tile([D_IN, B2_COLS[1] - B2_COLS[0]], F16, tag="xb2")
            wbb_sb = consts.tile([104, D_HID + D_OUT + 4], BF16, tag="wbb")
            # DMA issue order is robust to per-ring FIFO serialization
            # (a ring's 2nd issue was observed blocking on the 1st DMA's
            # completion): sync ring xb0 -> wbb -> xb2 matches need order;
            # xb1 rides scalar's ring, issued before the dummy exp.
            nc.sync.dma_start(xb0[:], xw[:, 0:B0_COLS])
            nc.scalar.dma_start(xb1[:], xw[:, B1_COLS[0]:B1_COLS[1]])
            nc.sync.dma_start(wbb_sb[:], wbb[:])
            nc.sync.dma_start(xb2[:], xw[:, B2_COLS[0]:B2_COLS[1]])

            # Dummy exp pulls the ~2.7us ACT_TABLE_LOAD forward, done
            # before the first real exp (~9us).  memzero is a Copy-
            # activation via bitcast — no table needed.
            expd = consts.tile([1, 16], F32, tag="expd")
            nc.scalar.memzero(expd[:])
            nc.scalar.activation(expd[:], expd[:], Act.Exp)

            # Junk warmup matmuls: PE activity from ~5.5us (vector memset
            # feeds them right after program load) so the HAM clock gate
            # opens (~3.4us of activity) and stays open when the real
            # matmuls start at ~9us.  Measured: without these, 41/65
            # real matmuls run at the cold 1.2 GHz rate (+17% PE time).
            junk = consts.tile([D_IN, FDP + D_OUT], F16, tag="junk")
            nc.vector.memset(junk[:], 0.0)
            warm = ps0.tile([D_HID, FDP], F32, tag="p0", name="warm")
            for _ in range(8):
                nc.tensor.matmul(warm[:D_OUT], junk[:, FDP:FDP + D_OUT],
                                 junk[:, 0:FDP], start=True, stop=True)

            w0_sb = xb0[:, 0:D_HID]
            w1_sb = wbb_sb[:D_HID, :D_HID]
            w2_sb = wbb_sb[:D_HID, D_HID:D_HID + D_OUT]
            bias_f32 = wbb_sb[:, D_HID + D_OUT:D_HID + D_OUT + 4].bitcast(F32)
            nb1_sb = bias_f32[:D_HID, 0:1]
            ncb2d_sb = bias_f32[:104, 1:2]

            def xsrc(g):
                if g == 0:
                    return xb0, D_HID
                if B1_GROUPS[0] <= g <= B1_GROUPS[1]:
                    return xb1, D_HID + _pstarts[g] - B1_COLS[0]
                return xb2, D_HID + _pstarts[g] - B2_COLS[0]

            pair_state = {}
            pair_sb = {0: {}, 1: {}}   # layer -> pair idx -> e/t pair tiles

            def pair_tiles(lyr, g):
                """Pair-wide e/t tiles [96, 1024]; group g uses cols
                off:off+fd.  The t-min runs ONCE per pair over the full
                width (DVE 4x amortizes the fixed pass overhead)."""
                pr = pair_sb[lyr].setdefault(g // 2, {})
                if g % 2 == 0:
                    pr["e"] = sb.tile([D_HID, 2 * FDP], BF16,
                                      tag=f"e{lyr}", name=f"e{lyr}")
                    pr["t"] = sb.tile([D_HID, 2 * FDP], BF16,
                                      tag=f"t{lyr}", name=f"t{lyr}")
                return pr, (g % 2) * FDP

            def stage0_mm(g):
                fd = _pairs[g]
                xt, xo = xsrc(g)
                s = st.setdefault(g, {})
                s["p0"] = ps0.tile([D_HID, FDP], F32, tag="p0", name="p0")
                s["r0"] = sb.tile([D_HID, FDP], BF16, tag="r0", name="r0")
                s["pr0"], s["off0"] = pair_tiles(0, g)
                nc.tensor.matmul(s["p0"][:, :fd], w0_sb, xt[:, xo:xo + fd],
                                 start=True, stop=True)

            def stage1_mm(g):
                fd = _pairs[g]
                s = st[g]
                o0 = s["off0"]
                s["p1"] = ps1.tile([D_HID, FDP], F32, tag="p1", name="p1")
                s["r1"] = sb.tile([D_HID, FDP], BF16, tag="r1", name="r1")
                s["pr1"], s["off1"] = pair_tiles(1, g)
                nc.tensor.matmul(s["p1"][:, :fd], w1_sb,
                                 s.pop("r0")[:, :fd], start=True, stop=False)
                nc.tensor.matmul(s["p1"][:, :fd], w1_sb,
                                 s["pr0"]["t"][:, o0:o0 + fd],
                                 start=False, stop=True)

            def stage2_mm(g):
                fd = _pairs[g]
                s = st[g]
                if g % 2 == 0:
                    p2 = ps2.tile([104, FDP], F32, tag="p2")
                    pair_state[g // 2] = p2
                    rows = slice(0, D_OUT)
                else:
                    p2 = pair_state[g // 2]
                    rows = slice(64, 64 + D_OUT)
                o1 = s["off1"]
                nc.tensor.matmul(p2[rows, :fd], w2_sb, s.pop("r1")[:, :fd],
                                 start=True, stop=False)
                nc.tensor.matmul(p2[rows, :fd], w2_sb,
                                 s["pr1"]["t"][:, o1:o1 + fd],
                                 start=False, stop=True)

            def act0(g):
                fd = _pairs[g]
                s = st[g]
                o0 = s["off0"]
                nc.scalar.activation(s["pr0"]["e"][:, o0:o0 + fd],
                                     s["p0"][:, :fd], Act.Exp)
                if g in R0_ON_ACT:
                    nc.scalar.activation(s["r0"][:, :fd], s["p0"][:, :fd],
                                         Act.Relu)

            def act1(g):
                fd = _pairs[g]
                s = st[g]
                o1 = s["off1"]
                nc.scalar.activation(s["pr1"]["e"][:, o1:o1 + fd],
                                     s["p1"][:, :fd], Act.Exp, bias=nb1_sb)
                if g in R1_ON_ACT:
                    nc.scalar.activation(s["r1"][:, :fd], s["p1"][:, :fd],
                                         Act.Relu, bias=nb1_sb)

            def dve0_r(g):
                fd = _pairs[g]
                s = st[g]
                if g not in R0_ON_ACT:
                    nc.vector.tensor_scalar_max(s["r0"][:, :fd],
                                                s["p0"][:, :fd], 0.0)

            def dve1_r(g):
                fd = _pairs[g]
                s = st[g]
                if g not in R1_ON_ACT:
                    nc.vector.tensor_scalar(s["r1"][:, :fd], s["p1"][:, :fd],
                                            nb1_sb, 0.0, Alu.add, Alu.max)

            def dve0_t(g):
                # one pair-wide min per pair, after the odd group's exp
                s = st[g]
                s.pop("p0")
                if (g % 2 == 1) or (g == P - 1):
                    pr = s["pr0"]
                    w = (g % 2) * FDP + _pairs[g]
                    nc.vector.tensor_scalar_min(pr["t"][:, :w],
                                                pr["e"][:, :w], 1.0)

            def dve1_t(g):
                s = st[g]
                s.pop("p1")
                if (g % 2 == 1) or (g == P - 1):
                    pr = s["pr1"]
                    w = (g % 2) * FDP + _pairs[g]
                    nc.vector.tensor_scalar_min(pr["t"][:, :w],
                                                pr["e"][:, :w], 1.0)

            def out_drain(g):
                if not ((g % 2 == 1) or (g == P - 1)):
                    return
                fd = _pairs[g]
                st.pop(g - 1, None)
                st.pop(g, None)
                p2 = pair_state.pop(g // 2)
                nrows = 104 if g % 2 == 1 else D_OUT
                o = opool.tile([104, FDP], F16, tag="o")
                nc.vector.tensor_scalar_add(o[:nrows, :fd], p2[:nrows, :fd],
                                            ncb2d_sb[:nrows])
                kp = g // 2
                # pair 5 rides scalar's idle ring so neither of the last
                # two DMAs queues behind a fresh completion receipt on
                # its ring (ring issues were observed blocking ~1us on
                # the prior DMA's receipt)
                eng = nc.scalar if kp == 5 else nc.sync
                eng.dma_start(yT[:, kp * FDP:kp * FDP + fd], o[:, :fd])

            # 5-deep software-pipelined emission (stage1/stage2 trail by
            # 2 ticks each) so every cross-engine dependency chain
            # (exp0 -> t0 -> p1-mm -> exp1 ...) has 2 ticks of slack and
            # the tick period is engine-work-bound, not latency-bound.
            # Per tick: all matmuls first, then ACT passes, then DVE
            # passes with fresh dependencies late in each FIFO queue.
            for pp in range(P + 5):
                a, b, c = pp - 1, pp - 3, pp - 5
                if 0 <= a < P:
                    stage0_mm(a)
                if 0 <= b < P:
                    stage1_mm(b)
                if 0 <= c < P:
                    stage2_mm(c)
                if 0 <= a < P:
                    act0(a)
                if 0 <= b < P:
                    act1(b)
                if 0 <= a < P:
                    dve0_r(a)
                if 0 <= b < P:
                    dve1_r(b)
                if 0 <= a < P:
                    dve0_t(a)
                if 0 <= b < P:
                    dve1_t(b)
                if 0 <= c < P:
                    out_drain(c)

    nc.compile()
    return nc


_prog_cache = []
last_result = None


def kernel(**inputs) -> np.ndarray:
    global last_result
    x = np.asarray(inputs["x"], np.float32)           # [50000, 128]
    W0 = np.asarray(inputs["W0"], np.float32).reshape(D_HID, D_IN)
    W1 = np.asarray(inputs["W1"], np.float32).reshape(D_HID, D_HID)
    W2 = np.asarray(inputs["W2"], np.float32).reshape(D_OUT, D_HID)

    n = x.shape[0]
    assert n == N_CORES * N_PER, f"unexpected node count {n}"

    import ml_dtypes
    xT16 = x.T.astype(np.float16)                            # [128, 50000]
    w0t = W0.T.astype(np.float16)                            # [128, 96]
    w1tb = W1.T.astype(ml_dtypes.bfloat16)                   # [96, 96]
    w2tb = W2.T.astype(ml_dtypes.bfloat16)                   # [96, 40]
    biasm = np.zeros((104, 2), np.float32)
    biasm[:D_HID, 0] = -w1tb.astype(np.float32).sum(axis=0)  # -(W1 @ 1)
    ncb2 = -w2tb.astype(np.float32).sum(axis=0)              # -(W2 @ 1)
    biasm[:D_OUT, 1] = ncb2
    biasm[64:64 + D_OUT, 1] = ncb2                           # replicated
    wbbm = np.zeros((104, D_HID + D_OUT + 4), ml_dtypes.bfloat16)
    wbbm[:D_HID, :D_HID] = w1tb
    wbbm[:D_HID, D_HID:D_HID + D_OUT] = w2tb
    wbbm.view(np.uint16)[:, D_HID + D_OUT:] = \
        np.ascontiguousarray(biasm).view(np.uint16)

    if not _prog_cache:
        _prog_cache.append(_build_program())
    nc = _prog_cache[0]

    in_maps = []
    for i in range(N_CORES):
        xwi = np.ascontiguousarray(
            np.concatenate([w0t, xT16[:, i * N_PER:(i + 1) * N_PER]], axis=1))
        in_maps.append(dict(xw=xwi, wbb=wbbm))
    res = run_bass_kernel_spmd(nc, in_maps, list(range(N_CORES)))
    last_result = res
    out = np.empty((n, D_OUT), np.float32)
    for i in range(N_CORES):
        yt = np.asarray(res.results[i]["yT"], np.float32)  # [104, 3178]
        base = i * N_PER
        for kp in range((P + 1) // 2):
            c0 = kp * FDP
            g0 = 2 * kp
            w0_ = _pairs[g0]
            out[base + _pstarts[g0]:base + _pstarts[g0] + w0_] = \
                yt[0:D_OUT, c0:c0 + w0_].T
            if g0 + 1 < P:
                w1_ = _pairs[g0 + 1]
                out[base + _pstarts[g0 + 1]:base + _pstarts[g0 + 1] + w1_] = \
                    yt[64:64 + D_OUT, c0:c0 + w1_].T
    return out


if __name__ == "__main__":
    data = np.load("/tmp/gat_inputs.npz")
    y = kernel(**{k: data[k] for k in data.files})
    print("out", y.shape, y.dtype, "absmax", np.abs(y).max())
